# revision 1
# baseline (speedup 1.0000x reference)
"""EntropyRegularizedVQ forward on 8 Trainium2 NeuronCores (Bass/Tile).

Strategy (data-parallel over N, hint-conformant):
  - shard z_e rows 8 ways; replicate the [1024, 64] codebook.
  - per core: PE computes -2*z@cb.T (fp32 matmul, products only);
    ACT injects z_sq with a single fp32 rounding (Relu+bias fma, values > 0);
    GPSIMD/DVE adds e_sq (the second reference-matching 64-scale rounding);
    DVE tensor_reduce(min) + max_index give first-occurrence argmin exactly;
    GPSIMD dma_gather fetches codebook rows; DVE replicates the
    straight-through z + (z_q - z) arithmetic bitwise.
  - host: all-reduce of the scalar losses and the code-usage histogram
    (tiny: 8 partial shards -> fp64 sums / bincount / entropy).

The two 64-scale roundings replicate XLA's fp32 d2 = (z_sq - 2*m) + e_sq
bitwise-faithfully, so argmin indices (incl. tie-breaks) match the
reference exactly.
"""
import numpy as np

N_TOKENS = 131072
NUM_CODES = 1024
EMBED_DIM = 64
N_CORES = 8
ROWS = N_TOKENS // N_CORES          # 16384 rows per core
GATHER_GROUP = 8
COMMITMENT_COST = 0.25
ENTROPY_WEIGHT = 0.1

_CACHE = {}


def _build(rows: int):
    import concourse.bacc as bacc
    import concourse.mybir as mybir
    from concourse.tile import TileContext

    D, K = EMBED_DIM, NUM_CODES
    f32 = mybir.dt.float32
    T = rows // 128
    GG = GATHER_GROUP
    n_groups = T // GG

    nc = bacc.Bacc()
    z_d = nc.declare_dram_parameter("z", [rows, D], f32, isOutput=False)
    cbm_d = nc.declare_dram_parameter("cbm", [D, K], f32, isOutput=False)
    esq_d = nc.declare_dram_parameter("esq_bcast", [128, K], f32, isOutput=False)
    cb_d = nc.declare_dram_parameter("cb_raw", [K, D], f32, isOutput=False)
    ident_d = nc.declare_dram_parameter("identity", [128, 128], f32, isOutput=False)

    idx_d = nc.declare_dram_parameter("idx", [T, 128], mybir.dt.int32, isOutput=True)
    d2min_d = nc.declare_dram_parameter("d2min", [T, 128], f32, isOutput=True)
    zqst_d = nc.declare_dram_parameter("zqst", [rows, D], f32, isOutput=True)

    with TileContext(nc) as tc:
        with (
            tc.tile_pool(name="const", bufs=1) as constp,
            tc.tile_pool(name="zin", bufs=3) as zinp,
            tc.tile_pool(name="zt", bufs=3) as ztp,
            tc.tile_pool(name="r2", bufs=3) as r2p,
            tc.tile_pool(name="small", bufs=6) as smallp,
            tc.tile_pool(name="zq", bufs=2) as zqp,
            tc.tile_pool(name="psA", bufs=2, space="PSUM") as psA,
            tc.tile_pool(name="psB", bufs=2, space="PSUM") as psB,
        ):
            cbm = constp.tile([D, K], f32, tag="cbm")
            nc.sync.dma_start(out=cbm[:], in_=cbm_d[:])
            esq = constp.tile([128, K], f32, tag="esq")
            nc.sync.dma_start(out=esq[:], in_=esq_d[:])
            ident = constp.tile([128, 128], f32, tag="ident")
            nc.sync.dma_start(out=ident[:], in_=ident_d[:])

            z_r = z_d.rearrange("(g q p) d -> g p q d", p=128, q=GG)
            zqst_r = zqst_d.rearrange("(g q p) d -> g p q d", p=128, q=GG)

            for g in range(n_groups):
                zg = zinp.tile([128, GG, D], f32, tag="zg")
                nc.sync.dma_start(out=zg[:], in_=z_r[g])
                kidx_i16 = smallp.tile([128, GG], mybir.dt.int16, tag="k16")

                for qq in range(0, GG, 2):
                    pt = psB.tile([128, 128], f32, tag="tp")
                    nc.tensor.transpose(pt[:], zg[:, qq:qq + 2, :], ident[:])
                    zt = ztp.tile([64, 2, 128], f32, tag="zt")
                    nc.vector.tensor_copy(zt[:, 0, :], pt[0:64, :])
                    nc.vector.tensor_copy(zt[:, 1, :], pt[64:128, :])

                    for h in range(2):
                        q = qq + h
                        j = g * GG + q
                        sq_scr = smallp.tile([128, D], f32, tag="sqscr")
                        zsq = smallp.tile([128, 1], f32, tag="zsq")
                        nc.scalar.activation(sq_scr[:], zg[:, q, :],
                                             mybir.ActivationFunctionType.Square,
                                             accum_out=zsq[:])

                        mneg = psA.tile([128, K], f32, tag="mm")
                        for c in range(2):
                            nc.tensor.matmul(mneg[:, c * 512:(c + 1) * 512],
                                             zt[:, h, :],
                                             cbm[:, c * 512:(c + 1) * 512],
                                             start=True, stop=True)

                        r1 = r2p.tile([128, K], f32, tag="r1")
                        nc.scalar.activation(r1[:], mneg[:],
                                             mybir.ActivationFunctionType.Relu,
                                             bias=zsq[:], scale=1.0)

                        r2 = r2p.tile([128, K], f32, tag="r2")
                        nc.gpsimd.tensor_add(r2[:], r1[:], esq[:])

                        dmin = smallp.tile([128, 1], f32, tag="dmin")
                        nc.vector.tensor_reduce(out=dmin[:], in_=r2[:],
                                                axis=mybir.AxisListType.X,
                                                op=mybir.AluOpType.min)
                        nc.sync.dma_start(out=d2min_d[j, :], in_=dmin[:, 0])

                        dmin8 = smallp.tile([128, 8], f32, tag="dmin8")
                        nc.vector.tensor_copy(dmin8[:], dmin[:].to_broadcast([128, 8]))
                        ku = smallp.tile([128, 8], mybir.dt.uint32, tag="ku")
                        nc.vector.max_index(ku[:], dmin8[:], r2[:])
                        ki = smallp.tile([128, 1], mybir.dt.int32, tag="ki")
                        nc.vector.tensor_copy(ki[:], ku[:, 0:1].bitcast(mybir.dt.int32))
                        nc.sync.dma_start(out=idx_d[j, :], in_=ki[:, 0])
                        nc.vector.tensor_copy(kidx_i16[:, q:q + 1], ki[:])

                wrapped = smallp.tile([128, GG * 8], mybir.dt.int16, tag="wrap")
                for b in range(8):
                    nc.sync.dma_start(
                        out=wrapped[0:16, :].rearrange("p (q b) -> p q b", b=8)[:, :, b],
                        in_=kidx_i16[16 * b:16 * (b + 1), :])
                for blk in range(1, 8):
                    nc.sync.dma_start(out=wrapped[16 * blk:16 * (blk + 1), :],
                                      in_=wrapped[0:16, :])

                zq = zqp.tile([128, GG, D], f32, tag="zq")
                nc.gpsimd.dma_gather(
                    out_ap=zq[:], in_ap=cb_d[:], idxs_ap=wrapped[:],
                    num_idxs=GG * 128, num_idxs_reg=GG * 128, elem_size=D)

                for s in range(0, GG, 8):
                    t1 = zqp.tile([128, 8, D], f32, tag="t1")
                    nc.vector.tensor_sub(t1[:], zq[:, s:s + 8, :], zg[:, s:s + 8, :])
                    t2 = zqp.tile([128, 8, D], f32, tag="t2")
                    nc.vector.tensor_add(t2[:], t1[:], zg[:, s:s + 8, :])
                    nc.sync.dma_start(out=zqst_r[g][:, s:s + 8, :], in_=t2[:])

    nc.compile()
    return nc


def _get_nc():
    if "nc" not in _CACHE:
        _CACHE["nc"] = _build(ROWS)
    return _CACHE["nc"]


def kernel(z_e: np.ndarray, codebook: np.ndarray):
    from concourse.bass_utils import run_bass_kernel_spmd

    z_e = np.ascontiguousarray(np.asarray(z_e, dtype=np.float32))
    cb = np.ascontiguousarray(np.asarray(codebook, dtype=np.float32))
    assert z_e.shape == (N_TOKENS, EMBED_DIM) and cb.shape == (NUM_CODES, EMBED_DIM)

    nc = _get_nc()

    cbm = np.ascontiguousarray((-2.0 * cb.T).astype(np.float32))
    e_sq = (cb * cb).sum(-1).astype(np.float32)
    esq_bcast = np.ascontiguousarray(np.tile(e_sq[None, :], (128, 1)))
    ident = np.eye(128, dtype=np.float32)

    in_maps = []
    for c in range(N_CORES):
        in_maps.append({
            "z": z_e[c * ROWS:(c + 1) * ROWS],
            "cbm": cbm, "esq_bcast": esq_bcast,
            "cb_raw": cb, "identity": ident,
        })
    results = run_bass_kernel_spmd(nc, in_maps, core_ids=list(range(N_CORES))).results

    # gather/unshard + the host-side all-reduce of scalars & histogram
    indices = np.empty(N_TOKENS, np.int32)
    d2min = np.empty(N_TOKENS, np.float64)
    z_q_st = np.empty((N_TOKENS, EMBED_DIM), np.float32)
    for c, res in enumerate(results):
        sl = slice(c * ROWS, (c + 1) * ROWS)
        indices[sl] = res["idx"].reshape(-1)
        d2min[sl] = res["d2min"].reshape(-1).astype(np.float64)
        z_q_st[sl] = res["zqst"]

    codebook_loss = np.float32(d2min.sum() / (N_TOKENS * EMBED_DIM))
    commitment_loss = np.float32(COMMITMENT_COST * float(codebook_loss))

    counts = np.bincount(indices, minlength=NUM_CODES).astype(np.float64)
    avg_probs = counts / N_TOKENS + 1e-10
    entropy = -(avg_probs * np.log(avg_probs)).sum()
    entropy_loss = np.float32(-ENTROPY_WEIGHT * (entropy / np.log(NUM_CODES)))
    perplexity = np.float32(np.exp(entropy))

    return (z_q_st, indices, codebook_loss, commitment_loss,
            entropy_loss, perplexity)


# revision 4
# speedup vs baseline: 31.2371x; 31.2371x over previous
"""EntropyRegularizedVQ forward on 8 Trainium2 NeuronCores (Bass/Tile).

Strategy (data-parallel over N, hint-conformant):
  - shard z_e rows 8 ways; replicate the [1024, 64] codebook.
  - per core: PE computes -2*z@cb.T (fp32 matmul, products only);
    ACT injects z_sq with a single fp32 rounding (Relu+bias fma, values > 0);
    GPSIMD/DVE adds e_sq (the second reference-matching 64-scale rounding);
    DVE tensor_reduce(min) + max_index give first-occurrence argmin exactly;
    GPSIMD dma_gather fetches codebook rows; DVE replicates the
    straight-through z + (z_q - z) arithmetic bitwise.
  - host: all-reduce of the scalar losses and the code-usage histogram
    (tiny: 8 partial shards -> fp64 sums / bincount / entropy).

The two 64-scale roundings replicate XLA's fp32 d2 = (z_sq - 2*m) + e_sq
bitwise-faithfully, so argmin indices (incl. tie-breaks) match the
reference exactly.
"""
import numpy as np

N_TOKENS = 131072
NUM_CODES = 1024
EMBED_DIM = 64
N_CORES = 8
ROWS = N_TOKENS // N_CORES          # 16384 rows per core
GATHER_GROUP = 8
COMMITMENT_COST = 0.25
ENTROPY_WEIGHT = 0.1

_CACHE = {}


def _build(rows: int):
    import concourse.bacc as bacc
    import concourse.mybir as mybir
    from concourse.tile import TileContext

    D, K = EMBED_DIM, NUM_CODES
    f32 = mybir.dt.float32
    T = rows // 128
    GG = GATHER_GROUP
    n_groups = T // GG

    nc = bacc.Bacc()
    z_d = nc.declare_dram_parameter("z", [rows, D], f32, isOutput=False)
    cbm_d = nc.declare_dram_parameter("cbm", [D, K], f32, isOutput=False)
    esq_d = nc.declare_dram_parameter("esq_bcast", [128, K], f32, isOutput=False)
    cb_d = nc.declare_dram_parameter("cb_raw", [K, D], f32, isOutput=False)
    ident_d = nc.declare_dram_parameter("identity", [128, 128], f32, isOutput=False)

    idx_d = nc.declare_dram_parameter("idx", [T, 128], mybir.dt.int32, isOutput=True)
    d2min_d = nc.declare_dram_parameter("d2min", [T, 128], f32, isOutput=True)
    zqst_d = nc.declare_dram_parameter("zqst", [rows, D], f32, isOutput=True)

    with TileContext(nc) as tc:
        with (
            tc.tile_pool(name="const", bufs=1) as constp,
            tc.tile_pool(name="zin", bufs=3) as zinp,
            tc.tile_pool(name="zt", bufs=3) as ztp,
            tc.tile_pool(name="r2", bufs=3) as r2p,
            tc.tile_pool(name="small", bufs=6) as smallp,
            tc.tile_pool(name="zq", bufs=2) as zqp,
            tc.tile_pool(name="psA", bufs=2, space="PSUM") as psA,
            tc.tile_pool(name="psB", bufs=2, space="PSUM") as psB,
        ):
            cbm = constp.tile([D, K], f32, tag="cbm")
            nc.sync.dma_start(out=cbm[:], in_=cbm_d[:])
            esq = constp.tile([128, K], f32, tag="esq")
            nc.sync.dma_start(out=esq[:], in_=esq_d[:])
            ident = constp.tile([128, 128], f32, tag="ident")
            nc.sync.dma_start(out=ident[:], in_=ident_d[:])

            z_r = z_d.rearrange("(g q p) d -> g p q d", p=128, q=GG)
            zqst_r = zqst_d.rearrange("(g q p) d -> g p q d", p=128, q=GG)

            for g in range(n_groups):
                zg = zinp.tile([128, GG, D], f32, tag="zg")
                nc.sync.dma_start(out=zg[:], in_=z_r[g])
                kidx_i16 = smallp.tile([128, GG], mybir.dt.int16, tag="k16")

                for qq in range(0, GG, 2):
                    pt = psB.tile([128, 128], f32, tag="tp")
                    nc.tensor.transpose(pt[:], zg[:, qq:qq + 2, :], ident[:])
                    zt = ztp.tile([64, 2, 128], f32, tag="zt")
                    nc.vector.tensor_copy(zt[:, 0, :], pt[0:64, :])
                    nc.vector.tensor_copy(zt[:, 1, :], pt[64:128, :])

                    for h in range(2):
                        q = qq + h
                        j = g * GG + q
                        sq_scr = smallp.tile([128, D], f32, tag="sqscr")
                        zsq = smallp.tile([128, 1], f32, tag="zsq")
                        nc.scalar.activation(sq_scr[:], zg[:, q, :],
                                             mybir.ActivationFunctionType.Square,
                                             accum_out=zsq[:])

                        mneg = psA.tile([128, K], f32, tag="mm")
                        for c in range(2):
                            nc.tensor.matmul(mneg[:, c * 512:(c + 1) * 512],
                                             zt[:, h, :],
                                             cbm[:, c * 512:(c + 1) * 512],
                                             start=True, stop=True)

                        r1 = r2p.tile([128, K], f32, tag="r1")
                        nc.scalar.activation(r1[:], mneg[:],
                                             mybir.ActivationFunctionType.Relu,
                                             bias=zsq[:], scale=1.0)

                        r2 = r2p.tile([128, K], f32, tag="r2")
                        nc.gpsimd.tensor_add(r2[:], r1[:], esq[:])

                        dmin = smallp.tile([128, 1], f32, tag="dmin")
                        nc.vector.tensor_reduce(out=dmin[:], in_=r2[:],
                                                axis=mybir.AxisListType.X,
                                                op=mybir.AluOpType.min)
                        nc.sync.dma_start(out=d2min_d[j, :], in_=dmin[:, 0])

                        dmin8 = smallp.tile([128, 8], f32, tag="dmin8")
                        nc.vector.tensor_copy(dmin8[:], dmin[:].to_broadcast([128, 8]))
                        ku = smallp.tile([128, 8], mybir.dt.uint32, tag="ku")
                        nc.vector.max_index(ku[:], dmin8[:], r2[:])
                        ki = smallp.tile([128, 1], mybir.dt.int32, tag="ki")
                        nc.vector.tensor_copy(ki[:], ku[:, 0:1].bitcast(mybir.dt.int32))
                        nc.sync.dma_start(out=idx_d[j, :], in_=ki[:, 0])
                        nc.vector.tensor_copy(kidx_i16[:, q:q + 1], ki[:])

                wrapped = smallp.tile([128, GG * 8], mybir.dt.int16, tag="wrap")
                for b in range(8):
                    nc.sync.dma_start(
                        out=wrapped[0:16, :].rearrange("p (q b) -> p q b", b=8)[:, :, b],
                        in_=kidx_i16[16 * b:16 * (b + 1), :])
                for blk in range(1, 8):
                    nc.sync.dma_start(out=wrapped[16 * blk:16 * (blk + 1), :],
                                      in_=wrapped[0:16, :])

                zq = zqp.tile([128, GG, D], f32, tag="zq")
                nc.gpsimd.dma_gather(
                    out_ap=zq[:], in_ap=cb_d[:], idxs_ap=wrapped[:],
                    num_idxs=GG * 128, num_idxs_reg=GG * 128, elem_size=D)

                for s in range(0, GG, 8):
                    t1 = zqp.tile([128, 8, D], f32, tag="t1")
                    nc.vector.tensor_sub(t1[:], zq[:, s:s + 8, :], zg[:, s:s + 8, :])
                    t2 = zqp.tile([128, 8, D], f32, tag="t2")
                    nc.vector.tensor_add(t2[:], t1[:], zg[:, s:s + 8, :])
                    nc.sync.dma_start(out=zqst_r[g][:, s:s + 8, :], in_=t2[:])

    nc.compile()
    return nc


def _get_nc():
    if "nc" not in _CACHE:
        _CACHE["nc"] = _build(ROWS)
    return _CACHE["nc"]


class _Runner:
    """Jit the SPMD executable once; reuse across kernel() calls."""

    def __init__(self, nc):
        import jax
        from jax.sharding import Mesh, PartitionSpec
        from jax.experimental.shard_map import shard_map
        import concourse.mybir as mybir
        from concourse import bass2jax

        bass2jax.install_neuronx_cc_hook()
        self.jax = jax
        part_name = nc.partition_id_tensor.name if nc.partition_id_tensor else None
        in_names, out_names, out_avals, zero_shapes = [], [], [], []
        for alloc in nc.m.functions[0].allocations:
            if not isinstance(alloc, mybir.MemoryLocationSet):
                continue
            name = alloc.memorylocations[0].name
            if alloc.kind == "ExternalInput":
                if name == part_name:
                    continue
                in_names.append(name)
            elif alloc.kind == "ExternalOutput":
                out_names.append(name)
                shape = tuple(alloc.tensor_shape)
                dtype = mybir.dt.np(alloc.dtype)
                out_avals.append(jax.core.ShapedArray(shape, dtype))
                zero_shapes.append((shape, dtype))
        n_params = len(in_names)
        n_outs = len(out_avals)
        all_names = in_names + out_names
        if part_name is not None:
            all_names = all_names + [part_name]
        donate = tuple(range(n_params, n_params + n_outs))

        def _body(*args):
            operands = list(args)
            if part_name is not None:
                operands.append(bass2jax.partition_id_tensor())
            outs = bass2jax._bass_exec_p.bind(
                *operands,
                out_avals=tuple(out_avals),
                in_names=tuple(all_names),
                out_names=tuple(out_names),
                lowering_input_output_aliases=(),
                sim_require_finite=True,
                sim_require_nnan=True,
                nc=nc,
            )
            return tuple(outs)

        devices = jax.devices()[:N_CORES]
        mesh = Mesh(np.asarray(devices), ("core",))
        specs = (PartitionSpec("core"),) * (n_params + n_outs)
        self.sharded = jax.jit(
            shard_map(_body, mesh=mesh, in_specs=specs,
                      out_specs=(PartitionSpec("core"),) * n_outs,
                      check_rep=False),
            donate_argnums=donate, keep_unused=True)
        self.in_names = in_names
        self.out_names = out_names
        self.out_avals = out_avals
        self.zero_shapes = zero_shapes

    def run(self, in_maps):
        concat_in = [
            np.concatenate([np.asarray(m[name]) for m in in_maps], axis=0)
            for name in self.in_names
        ]
        concat_zeros = [np.zeros((N_CORES * s[0], *s[1:]), d)
                        for s, d in self.zero_shapes]
        out_arrs = self.sharded(*concat_in, *concat_zeros)
        self.jax.block_until_ready(out_arrs)
        return [
            {name: np.asarray(out_arrs[i]).reshape(N_CORES, *self.out_avals[i].shape)[c]
             for i, name in enumerate(self.out_names)}
            for c in range(N_CORES)
        ]


def _get_runner():
    if "runner" not in _CACHE:
        _CACHE["runner"] = _Runner(_get_nc())
    return _CACHE["runner"]


def kernel(z_e: np.ndarray, codebook: np.ndarray):
    z_e = np.ascontiguousarray(np.asarray(z_e, dtype=np.float32))
    cb = np.ascontiguousarray(np.asarray(codebook, dtype=np.float32))
    assert z_e.shape == (N_TOKENS, EMBED_DIM) and cb.shape == (NUM_CODES, EMBED_DIM)

    cbm = np.ascontiguousarray((-2.0 * cb.T).astype(np.float32))
    e_sq = (cb * cb).sum(-1).astype(np.float32)
    esq_bcast = np.ascontiguousarray(np.tile(e_sq[None, :], (128, 1)))
    ident = np.eye(128, dtype=np.float32)

    in_maps = []
    for c in range(N_CORES):
        in_maps.append({
            "z": z_e[c * ROWS:(c + 1) * ROWS],
            "cbm": cbm, "esq_bcast": esq_bcast,
            "cb_raw": cb, "identity": ident,
        })
    results = _get_runner().run(in_maps)

    # gather/unshard + the host-side all-reduce of scalars & histogram
    indices = np.empty(N_TOKENS, np.int32)
    d2min = np.empty(N_TOKENS, np.float64)
    z_q_st = np.empty((N_TOKENS, EMBED_DIM), np.float32)
    for c, res in enumerate(results):
        sl = slice(c * ROWS, (c + 1) * ROWS)
        indices[sl] = res["idx"].reshape(-1)
        d2min[sl] = res["d2min"].reshape(-1).astype(np.float64)
        z_q_st[sl] = res["zqst"]

    codebook_loss = np.float32(d2min.sum() / (N_TOKENS * EMBED_DIM))
    commitment_loss = np.float32(COMMITMENT_COST * float(codebook_loss))

    counts = np.bincount(indices, minlength=NUM_CODES).astype(np.float64)
    avg_probs = counts / N_TOKENS + 1e-10
    entropy = -(avg_probs * np.log(avg_probs)).sum()
    entropy_loss = np.float32(-ENTROPY_WEIGHT * (entropy / np.log(NUM_CODES)))
    perplexity = np.float32(np.exp(entropy))

    return (z_q_st, indices, codebook_loss, commitment_loss,
            entropy_loss, perplexity)
